# revision 79
# baseline (speedup 1.0000x reference)
"""Trainium2 Bass kernel for nn_Attention_4363686773373.

Sigmoid attention with magnitude-preserving (weight-normalized) projections.

Sharding: data-parallel over (batch, T-half) -> 8 shards on 8 NeuronCores.
Each core computes q for its 1024 tokens and k,v for the full 2048 tokens of
its batch (k/v recomputed on both cores; no collectives). Each core's xkv
rows are pre-ordered so its query tokens come first (attention is
permutation-invariant over the key axis), keeping the program SPMD-uniform.

Per-core dataflow (heavy matmuls in bf16 with fp32 PSUM accumulation):
  W/X: row-normalize qkv_w on device and PE-transpose it (and the bf16-cast
     x) into [d, .] layouts; out_w is normalized and bounced through DRAM
     with large DMA-xbar transposes (only needed by the out-projection);
     per-token ||x|| via ACT square+accumulate.
  A: qkv projection in natural [t, e] layout (lhsT = xT tiles), fast PSUM
     eviction through ACT copies, q/k cosine-normalization along head_dim
     via free-dim reduces, then DRAM-bounce transposes to [head_dim, t]
     layout. q token-blocks are interleaved into the k/v loop.
  B: unit = (head-pair, t-half, key-block), t-half-major so early units only
     need early qn transposes. Both heads' K=64 score matmuls land side by
     side in one [128, 1024] PSUM tile (adjacent issue -> concurrent in
     disjoint PE row groups); ONE FD=1024 sigmoid on the scalar engine
     yields bf16 attn weights; attn^T @ v accumulates per pair. Score tiles
     are triple-buffered and issued two units ahead so the PE never stalls
     inside a sigmoid (keeps the PE HAM clock warm -- the biggest lever
     observed: a cold-entering phase B stays at half clock for its entire
     duration). Attn-out is evicted and xbar-transposed per t-half as soon as
     each half completes, on the idle sync ring.
  C: software-pipelined per token-block: normalize per (token, head), scale
     by token magnitude, PE-transpose to [d, t], out-projection, store.

DMA rings: sync = loads + xbar transposes, scalar = weight loads, gpsimd =
DRAM scratch writes + output stores. ACT table sets: all sqrt-set work
strictly precedes the sigmoid phase; one switch back for phase C's sqrt
(2 table switches total). PE-transpose batches land in a single-bank PSUM
tile and are evicted by one strided DVE copy. Measured ~416 us on 8
axon-tunneled trn2 cores, rel err 4.5e-3 vs the fp32 reference.
"""

import math
from contextlib import ExitStack

import numpy as np

import concourse.bass as bass
import concourse.tile as tile
from concourse import bacc, mybir
from concourse.bass_utils import run_bass_kernel_spmd
from concourse.masks import make_identity

# Problem shapes (hardcoded per harness contract)
B, T, D, H = 4, 2048, 768, 12
HD = D // H  # 64
EPS = 1e-4
SIGMOID_GAIN = 1.8402
N_CORES = 8

F32 = mybir.dt.float32
BF16 = mybir.dt.bfloat16
AF = mybir.ActivationFunctionType
ALU = mybir.AluOpType
AX = mybir.AxisListType


def _ensure_axon_hooks():
    """This image's antenv lacks axon_hooks; reconstruct it so trace=True
    (NTFF profiling) works instead of crashing on import."""
    try:
        import antenv.axon_hooks  # noqa: F401
        return
    except ImportError:
        pass
    import sys
    import types
    try:
        import antenv
    except ImportError:
        return
    mod = types.ModuleType("antenv.axon_hooks")
    _hook = [None]
    mod.set_axon_ntff_profile_hook = lambda h: _hook.__setitem__(0, h)
    mod.get_axon_ntff_profile_hook = lambda: _hook[0]
    sys.modules["antenv.axon_hooks"] = mod
    antenv.axon_hooks = mod
    try:
        from trn_agent_boot.trn_boot import _ntff_profile_via_ctypes
        mod.set_axon_ntff_profile_hook(
            _ntff_profile_via_ctypes('/opt/axon/libaxon_pjrt.so'))
    except Exception:
        pass


_ensure_axon_hooks()

if __import__("os").environ.get("ANT_LDW_OPT") == "1":
    import concourse.bass_utils as _bu
    _orig_rc = _bu.run_command

    def _rc_ldw(argv, **kw):
        argv = ["--enable-ldw-opt=true" if a == "--enable-ldw-opt=false" else a
                for a in argv]
        return _orig_rc(argv, **kw)

    _bu.run_command = _rc_ldw


def _chunks(total, maxn=1024):
    out = []
    c0 = 0
    while c0 < total:
        cn = min(maxn, total - c0)
        out.append((c0, cn))
        c0 += cn
    return out


def build_program(nc, tc, ctx, Tq, Tkv, Dm, Hn):
    """Emit the per-core program. xkv rows are pre-ordered so the first Tq
    tokens are this core's query tokens (attention is permutation-invariant
    over the key axis)."""
    keep = []  # keep tc.tile free-closures alive (GC would release the pools)

    def _tile(shape, dtype, name):
        t, free = tc.tile(shape, dtype, name=name)
        keep.append(free)
        return t, free

    tc._ant_keepalive = keep
    P = 128
    HDl = 64
    assert Dm % P == 0 and Tq % P == 0 and Tkv % P == 0
    DT = Dm // P          # d-tiles
    E3 = 3 * Dm
    PAIRS = Hn // 2       # head pairs; pair = 128 contiguous features
    assert PAIRS * P == Dm and Hn * HDl == Dm
    TBq = Tq // P
    TBkv = Tkv // P
    WE = E3 // P          # qkv_w row tiles
    # eps seen by the post-attention normalize, after folding out the
    # gain/sqrt(T) prefactor (we accumulate raw attn@v).
    eps_av = EPS * math.sqrt(Tkv) / SIGMOID_GAIN

    xkv = nc.dram_tensor("xkv", [Tkv, Dm], F32, kind="ExternalInput").ap()
    wT = nc.dram_tensor("wT", [Dm, E3], F32, kind="ExternalInput").ap()
    owT = nc.dram_tensor("owT", [Dm, Dm], F32, kind="ExternalInput").ap()
    y = nc.dram_tensor("y", [Tq, Dm], F32, kind="ExternalOutput").ap()

    # ---------------- DRAM scratch ----------------
    dstk = ExitStack()
    dpool = dstk.enter_context(tc.tile_pool(name="dram", bufs=1, space="DRAM"))
    kn_dram = dpool.tile([Tkv, Dm], BF16, name="kn_dram")
    qn_dram = dpool.tile([Tq, Dm], BF16, name="qn_dram")
    x_dram = dpool.tile([Tkv, Dm], BF16, name="x_dram")

    # ---------------- persistent SBUF tensors ----------------
    knT, _ = _tile([P, PAIRS * Tkv], BF16, "knT")    # [hd(2 heads), s]
    qnT, _ = _tile([P, PAIRS * Tq], BF16, "qnT")     # [hd(2 heads), t]
    vbig, _ = _tile([P, TBkv * Dm], BF16, "vbig")    # natural [s, e], raw
    mag8, _ = _tile([P, max(TBq, 2)], F32, "mag8")   # sqrt(||x||^2*HD/D)
    owTb, _ = _tile([P, DT * Dm], BF16, "owTb")      # raw out_w^T bf16
    avnat, _ = _tile([P, TBq * Dm], BF16, "avnat")   # attn-out natural
    ident, _ = _tile([P, P], BF16, "ident")          # PE-transpose identity
    make_identity(nc, ident)
    # weight-row recip norms (weight normalization is fused into PSUM
    # evictions instead of materializing normalized weights):
    rwb_q, _ = _tile([P, Dm], F32, "rwb_q")          # bcast 1/|wq_row|
    rwb_k, _ = _tile([P, Dm], F32, "rwb_k")          # bcast 1/|wk_row|
    rwb_ow, _ = _tile([P, Dm], F32, "rwb_ow")        # bcast 1/|ow_row|
    rwc_v, _ = _tile([P, DT], F32, "rwc_v")          # v recips, column form
    nslab, _ = _tile([P, Dm], BF16, "nslab")         # masked-gram scratch
    ones, _ = _tile([P, P], BF16, "ones")            # colsum-broadcast matmul
    nc.vector.memset(ones, 1.0)

    # ---------------- phase W + X + A (scoped) ----------------
    wxa = ExitStack()
    wTb, free_wTb = _tile([P, DT * E3], BF16, "wTb")      # raw qkv_w^T bf16
    xkvT, free_xkvT = _tile([P, DT * Tkv], BF16, "xkvT")
    wstage = wxa.enter_context(tc.tile_pool(name="wstage", bufs=6))
    sqpool = wxa.enter_context(tc.tile_pool(name="sqpool", bufs=4))
    small = wxa.enter_context(tc.tile_pool(name="small", bufs=24))
    nstage = wxa.enter_context(tc.tile_pool(name="nstage", bufs=6))
    psA = wxa.enter_context(tc.tile_pool(name="psA", bufs=2, space="PSUM"))
    psW = wxa.enter_context(tc.tile_pool(name="psW", bufs=1, space="PSUM"))
    psG = wxa.enter_context(tc.tile_pool(name="psG", bufs=1, space="PSUM"))

    def pe_transpose_cols(src, dst_big, cols, stride, base):
        """PE-transpose src [P, DT*P] column blocks into dst_big where block
        dt lands at dst_big[:, dt*stride + base : +cols]. All DT transposes
        land in one single-bank PSUM tile, evicted by ONE strided DVE copy."""
        ptw = psW.tile([P, DT * P], BF16, name="ptw", tag="ptw")
        for dt in range(DT):
            nc.tensor.transpose(ptw[:, dt * P:(dt + 1) * P],
                                src[:, dt * P:(dt + 1) * P], ident,
                                )
        dst3 = dst_big.rearrange("p (dt s) -> p dt s", dt=DT)[:, :, base:base + cols]
        nc.vector.tensor_copy(dst3, ptw.rearrange("p (dt s) -> p dt s", dt=DT))

    def load_w_cols(c0):
        """DMA qkv_w^T columns [c0, c0+Dm) of every d-tile (scalar ring) and
        ACT-cast to bf16 into wTb (raw, un-normalized)."""
        for dt in range(DT):
            wst = wstage.tile([P, Dm], F32, name="wst", tag="wst")
            nc.scalar.dma_start(wst, wT[dt * P:(dt + 1) * P, c0:c0 + Dm])
            nc.scalar.activation(
                wTb[:, dt * E3 + c0: dt * E3 + c0 + Dm], wst, AF.Copy)

    GB = min(4, DT)

    def gram_group(src_big, stride, cols, s0):
        """nslab[p, (s0+i)*P+q] = row_norm2(cols[i]*P+q) * (p == q): gram
        matmuls accumulated over d-tiles, diag isolated by one strided DVE
        multiply with the identity."""
        n = len(cols)
        gp = psG.tile([P, GB * P], F32, name="gp", tag="gp")
        for idx, col in enumerate(cols):
            for dt in range(DT):
                sl = src_big[:, dt * stride + col * P:
                             dt * stride + (col + 1) * P]
                nc.tensor.matmul(gp[:, idx * P:(idx + 1) * P], lhsT=sl,
                                 rhs=sl, start=(dt == 0), stop=(dt == DT - 1))
        nc.vector.tensor_tensor(
            nslab[:, s0 * P:(s0 + n) * P].rearrange("p (g q) -> p g q", g=n),
            gp[:, 0:n * P].rearrange("p (g q) -> p g q", g=n),
            ident.unsqueeze(1).broadcast_to([P, n, P]),
            op=ALU.mult)

    def gram_slab(src_big, stride, col0):
        for g0 in range(0, DT, GB):
            gn = min(GB, DT - g0)
            gram_group(src_big, stride, [col0 + g0 + i for i in range(gn)], g0)

    def recip_chain(dst):
        """dst[p, e] = 1/sqrt(colsum(nslab)[e]) broadcast over p, via a
        ones-matrix matmul (column sums broadcast to every out partition --
        ~200ns on the PE vs ~5us for a gpsimd partition_all_reduce). The +EPS
        of the reference is dropped: |w_row| ~ sqrt(Dm) >> EPS (err ~4e-6)."""
        t1 = wstage.tile([P, Dm], F32, name="t1", tag="wst")
        for (c0, cn) in _chunks(Dm, min(512, GB * P)):
            cs = psG.tile([P, GB * P], F32, name="cs", tag="gp")
            nc.tensor.matmul(cs[:, 0:cn], lhsT=ones, rhs=nslab[:, c0:c0 + cn],
                             start=True, stop=True)
            nc.scalar.activation(t1[:, c0:c0 + cn], cs[:, 0:cn], AF.Sqrt)
        nc.vector.reciprocal_approx_fast(out=dst, in_=t1)

    def load_x(ti):
        """x token-block ti: magnitude, bf16 cast, PE-transpose into xkvT."""
        xst = wstage.tile([P, Dm], F32, name="xst", tag="wst")
        nc.sync.dma_start(xst, xkv[ti * P:(ti + 1) * P, :])
        if ti < TBq:
            xsq = sqpool.tile([P, Dm], BF16, name="xsq", tag="sq")
            ssx = small.tile([P, 1], F32, name="ssx", tag="s1")
            nc.scalar.activation(xsq, xst, AF.Square, accum_out=ssx)
            nc.scalar.activation(mag8[:, ti:ti + 1], ssx, AF.Sqrt,
                                 scale=float(HDl) / float(Dm))
        xbf = nstage.tile([P, Dm], BF16, name="xbf", tag="nst")
        nc.vector.tensor_copy(xbf, xst)
        pe_transpose_cols(xbf, xkvT, P, Tkv, ti * P)

    # weight loads (k/v columns first -- the kv projection loop needs them),
    # early x blocks interleaved so the PE has transpose work immediately,
    # then the gram-norm chains.
    load_w_cols(Dm)          # k rows
    load_w_cols(2 * Dm)      # v rows
    for i in range(min(4, TBkv)):
        load_x(i)
    gram_slab(wTb, E3, DT)   # k
    recip_chain(rwb_k)
    gram_slab(wTb, E3, 2 * DT)  # v
    # v recips in column form [feature-in-pair, pair] for the phase-B
    # attn-out eviction (psav partitions are v features)
    vsq = small.tile([P, DT], F32, name="vsq", tag="svd")
    for j in range(DT):
        nc.vector.tensor_reduce(vsq[:, j:j + 1], nslab[:, j * P:(j + 1) * P],
                                axis=AX.X, op=ALU.add)
    vsr = small.tile([P, DT], F32, name="vsr", tag="svd")
    nc.scalar.activation(vsr, vsq, AF.Sqrt)
    nc.vector.reciprocal_approx_fast(out=rwc_v, in_=vsr)
    load_w_cols(0)           # q rows
    gram_slab(wTb, E3, 0)
    recip_chain(rwb_q)
    for i in range(min(4, TBkv), TBkv):
        load_x(i)
    # out-projection weights (gpsimd ring; only phase C needs them)
    for dt in range(DT):
        ost = wstage.tile([P, Dm], F32, name="ost", tag="wst")
        nc.gpsimd.dma_start(ost, owT[dt * P:(dt + 1) * P, :])
        nc.scalar.activation(owTb[:, dt * Dm:(dt + 1) * Dm], ost, AF.Copy)
    gram_slab(owTb, Dm, 0)
    recip_chain(rwb_ow)

    # qkv projection + q/k normalization, natural layout
    def qk_normalize(kraw, is_k):
        """kraw: SBUF bf16 [P, Dm] w-normalized q or k; returns cosine-
        normalized bf16 tile."""
        sqk = sqpool.tile([P, Dm], BF16, name="sqk", tag="sq")
        nc.scalar.activation(sqk, kraw, AF.Square)
        ssk = small.tile([P, Hn], F32, name="ssk", tag="sh")
        nc.vector.tensor_reduce(ssk, sqk.rearrange("p (h d) -> p h d", h=Hn),
                                axis=AX.X, op=ALU.add)
        # 1/(||q||+eps) -> 1/||q||: ||q|| ~ sqrt(HD) >> eps (error ~1e-5).
        # For k, fold the 1/sqrt(HD) score scale into the sqrt's free scale.
        sk = small.tile([P, Hn], F32, name="sk", tag="sh")
        nc.scalar.activation(sk, ssk, AF.Sqrt,
                             scale=(1.0 / HDl) if is_k else 1.0)
        rk = small.tile([P, Hn], F32, name="rk", tag="sh")
        nc.vector.reciprocal_approx_fast(out=rk, in_=sk)
        knb = nstage.tile([P, Dm], BF16, name="knb", tag="nst")
        nc.vector.tensor_tensor(
            knb.rearrange("p (h d) -> p h d", h=Hn),
            kraw.rearrange("p (h d) -> p h d", h=Hn),
            rk.broadcast_to([P, Hn, HDl]),
            op=ALU.mult)
        return knb

    def emit_q(ti):
        # q for this core's token blocks (first TBq blocks of xkv)
        ps = psA.tile([P, Dm], F32, name="psq", tag="ps")
        for dt in range(DT):
            lhs = xkvT[:, dt * Tkv + ti * P: dt * Tkv + (ti + 1) * P]
            for (c0, cn) in _chunks(Dm, 512):
                nc.tensor.matmul(ps[:, c0:c0 + cn], lhsT=lhs,
                                 rhs=wTb[:, dt * E3 + c0: dt * E3 + c0 + cn],
                                 start=(dt == 0), stop=(dt == DT - 1))
        qraw = sqpool.tile([P, Dm], BF16, name="qraw", tag="kraw")
        nc.vector.tensor_tensor(qraw, ps[:, 0:Dm], rwb_q, op=ALU.mult)
        qnb = qk_normalize(qraw, False)
        nc.gpsimd.dma_start(qn_dram[ti * P:(ti + 1) * P, :], qnb)
        QH = max(TBq // 2, 1)
        if ti % QH == QH - 1:
            h0 = (ti // QH) * QH * P
            hn = QH * P
            for pr in range(PAIRS):
                nc.sync.dma_start_transpose(
                    qnT[:, pr * Tq + h0: pr * Tq + h0 + hn],
                    qn_dram[h0:h0 + hn, pr * P:(pr + 1) * P])

    KQ = max(TBkv // 4, 1)
    qdone = 0
    for ti in range(TBkv):
        # k,v for every token block
        ps = psA.tile([P, 2 * Dm], F32, name="pskv", tag="ps")
        for dt in range(DT):
            lhs = xkvT[:, dt * Tkv + ti * P: dt * Tkv + (ti + 1) * P]
            for (c0, cn) in _chunks(2 * Dm, 512):
                nc.tensor.matmul(ps[:, c0:c0 + cn], lhsT=lhs,
                                 rhs=wTb[:, dt * E3 + Dm + c0: dt * E3 + Dm + c0 + cn],
                                 start=(dt == 0), stop=(dt == DT - 1))
        # evict PSUM: k with fused w-normalization (DVE), v raw (ACT copy;
        # its w-normalization is folded into the phase-B attn-out eviction)
        kraw = sqpool.tile([P, Dm], BF16, name="kraw", tag="kraw")
        nc.vector.tensor_tensor(kraw, ps[:, 0:Dm], rwb_k, op=ALU.mult)
        nc.scalar.activation(vbig[:, ti * Dm:(ti + 1) * Dm], ps[:, Dm:2 * Dm],
                             AF.Copy)
        knb = qk_normalize(kraw, True)
        nc.gpsimd.dma_start(kn_dram[ti * P:(ti + 1) * P, :], knb)
        if ti % KQ == KQ - 1:
            h0 = (ti // KQ) * KQ * P
            hn = KQ * P
            for pr in range(PAIRS):
                nc.sync.dma_start_transpose(
                    knT[:, pr * Tkv + h0: pr * Tkv + h0 + hn],
                    kn_dram[h0:h0 + hn, pr * P:(pr + 1) * P])
        # interleave q token-blocks so the PE stream stays dense into phase B
        qtarget = (ti + 1) * TBq // TBkv
        while qdone < qtarget:
            emit_q(qdone)
            qdone += 1

    wxa.close()
    free_xkvT()
    free_wTb()

    # ---------------- phase B: scores -> sigmoid -> attn @ v ----------------
    # Software-pipelined: scores for unit i+1 are issued to the PE before the
    # attn@v of unit i, so the PE works under each sigmoid instead of stalling
    # in FIFO order behind it. unit = (pair, key-block, head-in-pair).
    avt_big, _ = _tile([P, PAIRS * Tq], BF16, "avt_big")
    # pre-scale out_w^T by its row recips now (DVE is idle at phase-B entry)
    # so phase C's y eviction becomes a plain ACT copy off the busy DVE
    ownS, _ = _tile([P, DT * Dm], BF16, "ownS")
    for dt in range(DT):
        nc.vector.tensor_tensor(ownS[:, dt * Dm:(dt + 1) * Dm],
                                owTb[:, dt * Dm:(dt + 1) * Dm],
                                rwb_ow, op=ALU.mult)
    bstk = ExitStack()
    psS = bstk.enter_context(tc.tile_pool(name="psS", bufs=1, space="PSUM"))
    psAV = bstk.enter_context(tc.tile_pool(name="psAV", bufs=1, space="PSUM"))
    attnp = bstk.enter_context(tc.tile_pool(name="attnp", bufs=6))
    dvp = bstk.enter_context(tc.tile_pool(name="dvp", bufs=2))
    attnd = bstk.enter_context(tc.tile_pool(name="attnd", bufs=2))

    # unit = (pair, key-block, t-half). One [128, 1024] score tile holds BOTH
    # heads' [128, 512] score blocks side by side: the two K=64 matmuls are
    # emitted adjacently (concurrent in disjoint PE row groups), and ONE
    # FD=1024 sigmoid covers both heads.
    THW = min(512, Tq)
    TH = Tq // THW
    SW = 2 * THW
    units = [(pr, th, sb) for pr in range(PAIRS) for th in range(TH)
             for sb in range(TBkv)]
    n_units = len(units)
    psav_by_pair = {}
    # TWO score tiles: "big" holds two consecutive units side by side so ONE
    # FD=2048 sigmoid covers both (amortizing the ~350-cycle ACT overhead);
    # "small" holds the third unit of each period (FD=1024 sigmoid). Using
    # two whole tiles (not slots of one) keeps Tile's tile-granular
    # dependencies identical to the true deps: the fused sigmoid reads ALL
    # of big, so writers of either half must wait for it anyway.
    big = psS.tile([P, 2 * SW], F32, name="pbig", tag="big")
    sml = psS.tile([P, SW], F32, name="psml", tag="small")

    def emit_scores(i):
        pr, th, sb = units[i]
        r = i % 3
        pss = big[:, 0:SW] if r == 0 else (big[:, SW:2 * SW] if r == 1 else sml)
        for a in (0, 1):
            r0 = a * HDl
            nc.tensor.matmul(
                pss[:, a * THW:(a + 1) * THW],
                lhsT=knT[r0:r0 + HDl, pr * Tkv + sb * P: pr * Tkv + (sb + 1) * P],
                rhs=qnT[r0:r0 + HDl, pr * Tq + th * THW: pr * Tq + (th + 1) * THW],
                start=True, stop=True)

    # The sigmoid stream on ACT is phase B's critical path. For one key-block
    # per (pair, t-half) -- chosen to land on a "small"-tile position --
    # compute the sigmoid on the (otherwise idle) DVE instead: scores are
    # bounded (|q^.k^| <= 8), so
    #   sigmoid(x) = 1/(1 + e^-x) ~= 1/(1 + (1 - x/256)^256)
    # (8 squarings + approx-reciprocal, max abs err ~8e-4). Its attn@v is
    # deferred to just before the eviction so the in-order PE never waits on
    # the slower DVE chain.
    def _off_sb(pr, th):
        base = (pr * TH + th) * TBkv
        for sb in range(2, TBkv - 1):
            if (base + sb) % 3 == 2:
                return sb
        return -1

    off_map = {}
    if TBkv >= 4:
        for pr in range(PAIRS):
            for th in range(TH):
                s = _off_sb(pr, th)
                if s >= 0:
                    off_map[(pr, th)] = s

    def emit_av(u, attn, stop):
        pr, th, sb = u
        psav = psav_by_pair[pr]
        for a in (0, 1):
            r0 = a * HDl
            nc.tensor.matmul(
                psav[r0:r0 + HDl, th * THW:(th + 1) * THW],
                lhsT=vbig[:, sb * Dm + pr * P + r0: sb * Dm + pr * P + r0 + HDl],
                rhs=attn[:, a * THW:(a + 1) * THW],
                start=(sb == 0), stop=stop,
                skip_group_check=True)

    pending = {}
    attn_by_unit = {}

    def handle_unit(i):
        pr, th, sb = units[i]
        if sb == 0 and th == 0:
            psav_by_pair[pr] = psAV.tile([P, Tq], F32, name="psav", tag="psav")
        if off_map.get((pr, th)) == sb:
            pending[(pr, th)] = attn_by_unit.pop(i)
        else:
            stop = (sb == TBkv - 1) and ((pr, th) not in off_map)
            emit_av(units[i], attn_by_unit.pop(i), stop)
        if sb == TBkv - 1:
            if (pr, th) in off_map:
                emit_av((pr, th, off_map[(pr, th)]),
                        pending.pop((pr, th)), stop=True)
            finish_half(pr, th)

    def dve_sigmoid(i):
        za = dvp.tile([P, SW], F32, name="za", tag="dv")
        nc.vector.tensor_scalar(za, sml, -1.0 / 256.0, 1.0,
                                op0=ALU.mult, op1=ALU.add)
        zb = dvp.tile([P, SW], F32, name="zb", tag="dv")
        for _ in range(4):
            nc.vector.tensor_mul(zb, za, za)
            nc.vector.tensor_mul(za, zb, zb)
        nc.vector.tensor_scalar_add(zb, za, 1.0)
        nc.vector.reciprocal_approx_fast(out=za, in_=zb)
        attn = attnd.tile([P, SW], BF16, name="attnd", tag="attnd")
        nc.vector.tensor_copy(attn, za)
        attn_by_unit[i] = attn

    def finish_half(pr, th):
        # this t-half's attn-out is complete: evict it (fusing the v
        # weight-normalization: psav partition p is v-feature pr*128+p) and
        # run its natural-layout transposes on the idle sync xbar ring now,
        # shrinking the B->C boundary bubble
        psav = psav_by_pair[pr]
        c0 = th * THW
        nc.vector.tensor_tensor(
            avt_big[:, pr * Tq + c0: pr * Tq + c0 + THW],
            psav[:, c0:c0 + THW],
            rwc_v[:, pr:pr + 1].broadcast_to([P, THW]),
            op=ALU.mult)
        for tb in range(c0 // P, (c0 + THW) // P):
            nc.sync.dma_start_transpose(
                avnat[:, tb * Dm + pr * P: tb * Dm + (pr + 1) * P],
                avt_big[:, pr * Tq + tb * P: pr * Tq + (tb + 1) * P])

    for j in range(min(3, n_units)):
        emit_scores(j)
    for i in range(n_units):
        pr, th, sb = units[i]
        r = i % 3
        if r == 1 or (r == 0 and i == n_units - 1):
            # fused sigmoid over units (i-1, i) in big -- or a final lone
            # unit in big's first half
            m = 2 if r == 1 else 1
            at = attnp.tile([P, m * SW], BF16, name="attnF", tag=f"at{m}")
            nc.scalar.activation(at, big[:, 0:m * SW], AF.Sigmoid)
            if m == 2:
                attn_by_unit[i - 1] = at[:, 0:SW]
                attn_by_unit[i] = at[:, SW:2 * SW]
            else:
                attn_by_unit[i] = at
            # next scores into big go out BEFORE the avs so the following
            # fused sigmoid's inputs are produced with minimal ACT bubble
            if i + 2 < n_units:
                emit_scores(i + 2)
            if i + 3 < n_units:
                emit_scores(i + 3)
            if m == 2:
                handle_unit(i - 1)
            handle_unit(i)
        elif r == 2:
            upr, uth, usb = units[i]
            if off_map.get((upr, uth)) == usb:
                dve_sigmoid(i)
            else:
                at = attnp.tile([P, SW], BF16, name="attnS", tag="at1")
                nc.scalar.activation(at, sml, AF.Sigmoid)
                attn_by_unit[i] = at
            if i + 3 < n_units:
                emit_scores(i + 3)
            handle_unit(i)
    bstk.close()

    # ---------------- phase C: normalize + magnitude + out-proj ----------------
    avnT, _ = _tile([P, DT * Tq], BF16, "avnT")
    cstk = ExitStack()
    psO = cstk.enter_context(tc.tile_pool(name="psO", bufs=2, space="PSUM"))
    psT2 = cstk.enter_context(tc.tile_pool(name="psT2", bufs=4, space="PSUM"))
    sqc = cstk.enter_context(tc.tile_pool(name="sqc", bufs=4))
    smallc = cstk.enter_context(tc.tile_pool(name="smallc", bufs=24))
    avnp = cstk.enter_context(tc.tile_pool(name="avnp", bufs=4))
    ypool = cstk.enter_context(tc.tile_pool(name="ypool", bufs=3))

    def c_norm(tb):
        src = avnat[:, tb * Dm:(tb + 1) * Dm]
        sqa = sqc.tile([P, Dm], BF16, name="sqa", tag="sqa")
        nc.scalar.activation(sqa, src, AF.Square)
        ssa = smallc.tile([P, Hn], F32, name="ssa", tag="sh")
        nc.vector.tensor_reduce(ssa, sqa.rearrange("p (h d) -> p h d", h=Hn),
                                axis=AX.X, op=ALU.add)
        sa = smallc.tile([P, Hn], F32, name="sa", tag="sh")
        nc.scalar.activation(sa, ssa, AF.Sqrt)
        # 1/(||av||+eps_av) -> 1/||av||: ||av|| >> eps_av w.h.p. (~1e-5)
        ra = smallc.tile([P, Hn], F32, name="ra", tag="sh")
        nc.vector.reciprocal_approx_fast(out=ra, in_=sa)
        g = smallc.tile([P, Hn], F32, name="g", tag="sh")
        nc.vector.tensor_scalar_mul(g, ra, mag8[:, tb:tb + 1])
        avn = avnp.tile([P, Dm], BF16, name="avn", tag="avn")
        nc.vector.tensor_tensor(
            avn.rearrange("p (h d) -> p h d", h=Hn),
            src.rearrange("p (h d) -> p h d", h=Hn),
            g.broadcast_to([P, Hn, HDl]),
            op=ALU.mult)
        ptt = psT2.tile([P, DT * P], BF16, name="ptt2", tag="ptt2")
        for dt in range(DT):
            nc.tensor.transpose(ptt[:, dt * P:(dt + 1) * P],
                                avn[:, dt * P:(dt + 1) * P], ident)
        dst3 = avnT.rearrange("p (dt s) -> p dt s", dt=DT)[:, :, tb * P:(tb + 1) * P]
        nc.scalar.activation(dst3, ptt.rearrange("p (dt s) -> p dt s", dt=DT),
                             AF.Copy)

    def c_proj(tb):
        pso = psO.tile([P, Dm], F32, name="pso", tag="pso")
        for dt in range(DT):
            lhs = avnT[:, dt * Tq + tb * P: dt * Tq + (tb + 1) * P]
            for (c0, cn) in _chunks(Dm, 512):
                nc.tensor.matmul(pso[:, c0:c0 + cn], lhsT=lhs,
                                 rhs=ownS[:, dt * Dm + c0: dt * Dm + c0 + cn],
                                 start=(dt == 0), stop=(dt == DT - 1))
        ysb = ypool.tile([P, Dm], F32, name="ysb", tag="ysb")
        nc.scalar.activation(ysb, pso, AF.Copy)
        nc.gpsimd.dma_start(y[tb * P:(tb + 1) * P, :], ysb)

    for tb in range(TBq + 1):
        if tb < TBq:
            c_norm(tb)
        if tb >= 1:
            c_proj(tb - 1)
    cstk.close()
    dstk.close()


def make_nc(Tq=T // 2, Tkv=T, Dm=D, Hn=H):
    nc = bacc.Bacc("TRN2", target_bir_lowering=False, debug=False,
                   num_devices=N_CORES)
    with ExitStack() as ctx:
        with tile.TileContext(nc) as tc:
            build_program(nc, tc, ctx, Tq, Tkv, Dm, Hn)
    nc.compile()
    return nc


_CACHED_NC = None


def _get_nc():
    global _CACHED_NC
    if _CACHED_NC is None:
        _CACHED_NC = make_nc()
    return _CACHED_NC


def _shard_inputs(x, qkv_w, out_w):
    Tq = T // 2
    x = np.asarray(x, dtype=np.float32)
    wT = np.ascontiguousarray(np.asarray(qkv_w, dtype=np.float32).T)
    owT = np.ascontiguousarray(np.asarray(out_w, dtype=np.float32).T)
    in_maps = []
    for core in range(N_CORES):
        b, half = core // 2, core % 2
        own = x[b, half * Tq:(half + 1) * Tq]
        other = x[b, (1 - half) * Tq:(2 - half) * Tq]
        xkv = np.ascontiguousarray(np.concatenate([own, other], axis=0))
        in_maps.append({"xkv": xkv, "wT": wT, "owT": owT})
    return in_maps


def run(x, qkv_w, out_w, trace=False, trace_cores=None):
    nc = _get_nc()
    in_maps = _shard_inputs(x, qkv_w, out_w)
    res = run_bass_kernel_spmd(nc, in_maps, list(range(N_CORES)),
                               trace=trace, trace_cores=trace_cores)
    Tq = T // 2
    y = np.empty((B, T, D), np.float32)
    for core, r in enumerate(res.results):
        b, half = core // 2, core % 2
        y[b, half * Tq:(half + 1) * Tq] = r["y"]
    return y, res


def kernel(x, qkv_w, out_w):
    y, _ = run(x, qkv_w, out_w, trace=False)
    return y

